# revision 11
# baseline (speedup 1.0000x reference)
"""Multi-head attention (B=8, T=1024, d_model=1024, H=16, d=64) on 8 trn2 cores.

Strategy: data-parallel over batch — one batch element per NeuronCore, no
collectives. Per core, everything is computed in "transposed" layouts so that
every matmul has its contraction on the partition dim and a 512-wide moving
operand:

  qhT/khT = (Wq.T @ q.T) etc.            [hd, t]   (PE, bf16 in / fp32 psum)
  vh      = v @ Wv.T (+ ones column)     [t, hd]   per-head [tk, 64+1]
  S.T     = khT_h.T @ qhT_h              [tk, tq]  (K=64)
  E       = exp(S.T)                     (ACT, no max-subtract: logits O(6))
  ctx_ext = [vh | 1].T @ E               [65, tq]  row 64 = softmax denominators
  attn.T  = E * (1/denom)                (DVE 4x bf16, partition-broadcast recip)
  out.T   = Wo @ (ctx_ext[0:64]/denom)   [64, t]   fp32 out

attn is written to HBM as one contiguous [128, 4096] block per (head, tq-chunk)
job; the host unscrambles to (B,H,Tq,Tk) fp32. The matmul datapath is bf16
(fp32 PSUM accumulation) which enables FWL fast weight loads and 4x DVE modes.
"""
import os
import sys
from contextlib import ExitStack

import numpy as np
import ml_dtypes

for _p in ("/opt/trn_rl_repo", os.path.expanduser("~/.axon_site/_ro/trn_rl_repo")):
    if os.path.isdir(_p) and _p not in sys.path:
        sys.path.append(_p)

import concourse.bass as bass  # noqa: E402
import concourse.tile as tile  # noqa: E402
from concourse import bacc, mybir  # noqa: E402
from concourse.bass_utils import run_bass_kernel_spmd  # noqa: E402

F32 = mybir.dt.float32
BF16 = mybir.dt.bfloat16
AF = mybir.ActivationFunctionType
BF16_NP = ml_dtypes.bfloat16

B, T, DM, H, D = 8, 1024, 1024, 16, 64
HD = H * D
P = 128
NT = T // P     # 8 partition tiles along t
NK = DM // P    # 8 contraction tiles along d_model / hd
TQC = 512       # tq chunk (one PSUM bank)
NCH = T // TQC  # 2
NEG = -1.0e9
E_BUFS = 4      # merged exp tiles [128, 2*NT*TQC]: jobs in flight

LAST_RESULTS = None


def _act_recip_lnexp(nc, out, tmp, in_):
    """1/x as exp(-ln(x)) on ACT — both funcs live in the
    natural_log_exp_and_others table, so no ACT table reloads (unlike
    Reciprocal, which forces a table swap against Exp every job)."""
    nc.scalar.activation(tmp, in_, AF.Ln)
    nc.scalar.activation(out, tmp, AF.Exp, scale=-1.0)


def _build(with_mask, with_bq, with_bk, with_bv, with_bo):
    nc = bacc.Bacc("TRN2", target_bir_lowering=False, debug=False, num_devices=1)

    qT_d = nc.dram_tensor("qT", [DM, T], BF16, kind="ExternalInput")
    kT_d = nc.dram_tensor("kT", [DM, T], BF16, kind="ExternalInput")
    vT_d = nc.dram_tensor("vT", [DM, T], BF16, kind="ExternalInput")
    wqT_d = nc.dram_tensor("WqT", [DM, HD], BF16, kind="ExternalInput")
    wkT_d = nc.dram_tensor("WkT", [DM, HD], BF16, kind="ExternalInput")
    wvT_d = nc.dram_tensor("WvT", [DM, HD], BF16, kind="ExternalInput")
    woT_d = nc.dram_tensor("WoT", [HD, D], BF16, kind="ExternalInput")
    ones_d = nc.dram_tensor("ones_h", [P, H], BF16, kind="ExternalInput")
    bq_d = nc.dram_tensor("bq2", [HD, 1], F32, kind="ExternalInput") if with_bq else None
    bk_d = nc.dram_tensor("bk2", [HD, 1], F32, kind="ExternalInput") if with_bk else None
    bv_d = nc.dram_tensor("bv_ext", [1, H * 65], F32, kind="ExternalInput") if with_bv else None
    bo_d = nc.dram_tensor("bo2", [D, 1], F32, kind="ExternalInput") if with_bo else None
    maskT_d = nc.dram_tensor("maskT", [T, T], F32, kind="ExternalInput") if with_mask else None

    # per job (h, c): rows (h*NCH+c)*128 .. +128, cols = (kt, n) flattened
    attn_d = nc.dram_tensor("attn_s", [H * NCH * P, NT * TQC], BF16, kind="ExternalOutput")
    out_d = nc.dram_tensor("out_t", [D, T], F32, kind="ExternalOutput")

    with tile.TileContext(nc) as tc, ExitStack() as ctx, \
         nc.allow_low_precision(reason="bf16 attention datapath by design"):
        # ---- persistent pools -------------------------------------------
        qhT_p = ctx.enter_context(tc.tile_pool(name="qhT", bufs=NK))
        khT_p = ctx.enter_context(tc.tile_pool(name="khT", bufs=NK))
        vh_p = ctx.enter_context(tc.tile_pool(name="vh", bufs=NT))
        ctx_p = ctx.enter_context(tc.tile_pool(name="ctxp", bufs=NK))
        wo_p = ctx.enter_context(tc.tile_pool(name="wo", bufs=NK))
        sm_p = ctx.enter_context(tc.tile_pool(name="smallp", bufs=4))

        qhT = [qhT_p.tile([P, T], BF16, tag="qhT", name=f"qhT{i}") for i in range(NK)]
        khT = [khT_p.tile([P, T], BF16, tag="khT", name=f"khT{i}") for i in range(NK)]
        vh = [vh_p.tile([P, H * 65], BF16, tag="vh", name=f"vh{i}") for i in range(NT)]
        ctxT = [ctx_p.tile([P, T], BF16, tag="ctx", name=f"ctxT{i}") for i in range(NK)]
        wo_sb = [wo_p.tile([P, D], BF16, tag="wo", name=f"wo{i}") for i in range(NK)]
        for k in range(NK):
            nc.sync.dma_start(wo_sb[k][:], woT_d.ap()[k * P:(k + 1) * P, :])

        # ones columns of vh (slot 64 of each per-head 65-block)
        for i in range(NT):
            dst = vh[i][:, :].rearrange("p (h x) -> p h x", x=65)[:, :, 64:65]
            src = ones_d.ap()[:, :].rearrange("p (h o) -> p h o", o=1)
            nc.sync.dma_start(dst, src)

        bo_sb = None
        if with_bo:
            bo_sb = sm_p.tile([D, 1], F32, tag="bo")
            nc.sync.dma_start(bo_sb[:], bo_d.ap())

        # ---- phase P: projections ---------------------------------------
        with tc.tile_pool(name="wf", bufs=NK + 2) as w_p, \
             tc.tile_pool(name="xf", bufs=NK + 2) as x_p, \
             tc.tile_pool(name="psp", bufs=4, space="PSUM") as ps_p, \
             tc.tile_pool(name="biasp", bufs=2 * NK + 2) as bias_p:

            def load_full(pool, tag, d_tensor, pname):
                ts = []
                for k in range(NK):
                    t_ = pool.tile([P, T], BF16, tag=tag, name=f"{pname}_{k}")
                    nc.sync.dma_start(t_[:], d_tensor.ap()[k * P:(k + 1) * P, :])
                    ts.append(t_)
                return ts

            def proj_qk(w_d, x_d, dst_tiles, b_d, pname):
                w_sb = load_full(w_p, "wf", w_d, pname + "w")
                x_sb = load_full(x_p, "xf", x_d, pname + "x")
                b_sb = []
                if b_d is not None:
                    for i in range(NK):
                        bt = bias_p.tile([P, 1], F32, tag="bias", name=f"bias{pname}_{i}")
                        nc.sync.dma_start(bt[:], b_d.ap()[i * P:(i + 1) * P, :])
                        b_sb.append(bt)
                for i in range(NK):
                    for c in range(NCH):
                        ps = ps_p.tile([P, TQC], F32, tag="psp", name=f"ps{pname}_{i}_{c}")
                        for k in range(NK):
                            nc.tensor.matmul(
                                ps[:], w_sb[k][:, i * P:(i + 1) * P],
                                x_sb[k][:, c * TQC:(c + 1) * TQC],
                                start=(k == 0), stop=(k == NK - 1))
                        dst = dst_tiles[i][:, c * TQC:(c + 1) * TQC]
                        if b_d is not None:
                            nc.scalar.activation(dst, ps[:], AF.Identity, bias=b_sb[i][:])
                        else:
                            nc.scalar.activation(dst, ps[:], AF.Copy)

            proj_qk(wqT_d, qT_d, qhT, bq_d, "q")
            proj_qk(wkT_d, kT_d, khT, bk_d, "k")

            # v: vh[t, hd] with 65-stride per head; vT slices stationary,
            # WvT chunks moving.
            w_sb = load_full(w_p, "wf", wvT_d, "vw")
            x_sb = load_full(x_p, "xf", vT_d, "vx")
            bvb = None
            if with_bv:
                bv_row = bias_p.tile([1, H * 65], F32, tag="bvrow")
                nc.sync.dma_start(bv_row[:], bv_d.ap())
                bvb = bias_p.tile([P, H * 65], F32, tag="bvb")
                nc.gpsimd.partition_broadcast(bvb[:], bv_row[:])
            for i in range(NT):
                for c in range(NCH):
                    ps = ps_p.tile([P, TQC], F32, tag="psp", name=f"psv_{i}_{c}")
                    for k in range(NK):
                        nc.tensor.matmul(
                            ps[:], x_sb[k][:, i * P:(i + 1) * P],
                            w_sb[k][:, c * TQC:(c + 1) * TQC],
                            start=(k == 0), stop=(k == NK - 1))
                    dst = vh[i][:, :].rearrange("p (h x) -> p h x", x=65)[:, 8 * c:8 * (c + 1), 0:64]
                    src = ps[:, :].rearrange("p (h x) -> p h x", x=64)
                    nc.scalar.activation(dst, src, AF.Copy)
                if with_bv:
                    nc.vector.tensor_add(vh[i][:], vh[i][:], bvb[:])

        # ---- phase A: attention -----------------------------------------
        with ExitStack() as actx:
            e_p = actx.enter_context(tc.tile_pool(name="e", bufs=E_BUFS))
            b_p = actx.enter_context(tc.tile_pool(name="bb", bufs=3))
            r_p = actx.enter_context(tc.tile_pool(name="rc", bufs=4))
            ps_s = actx.enter_context(tc.tile_pool(name="pss", bufs=2, space="PSUM"))
            ps_c = actx.enter_context(tc.tile_pool(name="psc", bufs=3, space="PSUM"))
            maskT_sb = None
            if with_mask:
                m_p = actx.enter_context(tc.tile_pool(name="maskp", bufs=NT))
                maskT_sb = []
                for kt in range(NT):
                    mt = m_p.tile([P, T], F32, tag="mask", name=f"mask_{kt}")
                    nc.sync.dma_start(mt[:], maskT_d.ap()[kt * P:(kt + 1) * P, :])
                    maskT_sb.append(mt)

            NTQ = NT * TQC

            def scores(hp, c):
                # heads (2hp, 2hp+1) share khT/qhT tile hp; the two K=64
                # matmuls are row-packed (tile_position) and run concurrently,
                # writing the two halves of one 2-bank psum tile so a single
                # [128,1024] exp serves both heads.
                eb = e_p.tile([P, 2 * NTQ], BF16, tag="e", name=f"e_{hp}_{c}")
                for kt in range(NT):
                    sp = ps_s.tile([P, 2 * TQC], F32, tag="sp", name=f"sp_{hp}_{c}_{kt}")
                    for s_i in range(2):
                        nc.tensor.matmul(
                            sp[:, s_i * TQC:(s_i + 1) * TQC],
                            khT[hp][s_i * 64:(s_i + 1) * 64, kt * P:(kt + 1) * P],
                            qhT[hp][s_i * 64:(s_i + 1) * 64, c * TQC:(c + 1) * TQC],
                            start=True, stop=True, tile_position=(s_i * 64, 0))
                    if with_mask:
                        mrep = maskT_sb[kt][:, c * TQC:(c + 1) * TQC] \
                            .rearrange("p (o q) -> p o q", o=1).to_broadcast([P, 2, TQC])
                        nc.vector.tensor_add(
                            sp[:, :].rearrange("p (s q) -> p s q", s=2),
                            sp[:, :].rearrange("p (s q) -> p s q", s=2), mrep)
                    dst = eb[:, :].rearrange("p (s q) -> p s q", q=NTQ)[:, :, kt * TQC:(kt + 1) * TQC]
                    nc.scalar.activation(dst, sp[:, :].rearrange("p (s q) -> p s q", s=2), AF.Exp)
                return eb

            def tail(hp, c, eb):
                for s_i in range(2):
                    h = 2 * hp + s_i
                    off = s_i * NTQ
                    cp = ps_c.tile([D + 1, TQC], F32, tag="cp", name=f"cp_{h}_{c}")
                    for kt in range(NT):
                        nc.tensor.matmul(
                            cp[:], vh[kt][:, h * 65:(h + 1) * 65],
                            eb[:, off + kt * TQC:off + (kt + 1) * TQC],
                            start=(kt == 0), stop=(kt == NT - 1))
                    # 1/denominators straight off the psum row (ACT, bf16 out)
                    lt = r_p.tile([1, TQC], F32, tag="lt", name=f"lt_{h}_{c}")
                    rc = r_p.tile([1, TQC], BF16, tag="rc", name=f"rc_{h}_{c}")
                    _act_recip_lnexp(nc, rc[:], lt[:], cp[D:D + 1, :])
                    bb = b_p.tile([P, TQC], BF16, tag="bb", name=f"bb_{h}_{c}")
                    nc.gpsimd.partition_broadcast(bb[:], rc[:])
                    # ctx normalize (psum fp32 x bf16 -> bf16)
                    nc.vector.tensor_mul(
                        ctxT[hp][s_i * 64:(s_i + 1) * 64, c * TQC:(c + 1) * TQC],
                        cp[0:D, :], bb[0:D, :])
                    # attn normalize: one mul over the head's half of eb
                    bb_rep = bb[:, :].rearrange("p (k n) -> p k n", k=1).to_broadcast([P, NT, TQC])
                    nc.vector.tensor_mul(
                        eb[:, off:off + NTQ].rearrange("p (k n) -> p k n", n=TQC),
                        eb[:, off:off + NTQ].rearrange("p (k n) -> p k n", n=TQC), bb_rep)
                    row = (h * NCH + c) * P
                    nc.sync.dma_start(attn_d.ap()[row:row + P, :], eb[:, off:off + NTQ])

            jobs = [(hp, c) for hp in range(H // 2) for c in range(NCH)]
            prev = None
            for j in jobs:
                eb = scores(*j)
                if prev is not None:
                    tail(prev[0][0], prev[0][1], prev[1])
                prev = (j, eb)
            tail(prev[0][0], prev[0][1], prev[1])

        # ---- phase O: output projection ---------------------------------
        with tc.tile_pool(name="pso", bufs=2, space="PSUM") as ps_o, \
             tc.tile_pool(name="ot", bufs=1) as ot_p:
            outT = ot_p.tile([D, T], F32)
            for c in range(NCH):
                op = ps_o.tile([D, TQC], F32, tag="op", name=f"op_{c}")
                for k in range(NK):
                    nc.tensor.matmul(
                        op[:], wo_sb[k][:], ctxT[k][:, c * TQC:(c + 1) * TQC],
                        start=(k == 0), stop=(k == NK - 1))
                if with_bo:
                    nc.scalar.activation(outT[:, c * TQC:(c + 1) * TQC], op[:], AF.Identity, bias=bo_sb[:])
                else:
                    nc.scalar.activation(outT[:, c * TQC:(c + 1) * TQC], op[:], AF.Copy)
            nc.sync.dma_start(out_d.ap(), outT[:])

    nc.compile()
    return nc


_NC_CACHE = {}


def _get_nc(cfg):
    if cfg not in _NC_CACHE:
        _NC_CACHE[cfg] = _build(*cfg)
    return _NC_CACHE[cfg]


def kernel(q, k, v, Wq, bq, Wk, bk, Wv, bv, Wo, bo, mask):
    global LAST_RESULTS
    q = np.asarray(q, np.float32)
    k = np.asarray(k, np.float32)
    v = np.asarray(v, np.float32)
    Wq = np.asarray(Wq, np.float32)
    Wk = np.asarray(Wk, np.float32)
    Wv = np.asarray(Wv, np.float32)
    Wo = np.asarray(Wo, np.float32)
    bq = np.asarray(bq, np.float32)
    bk = np.asarray(bk, np.float32)
    bv = np.asarray(bv, np.float32)
    bo = np.asarray(bo, np.float32)
    mask = np.asarray(mask)
    assert q.shape == (B, T, DM) and k.shape == (B, T, DM) and v.shape == (B, T, DM)

    with_mask = bool((mask == 0).any())
    with_bq = bool(np.any(bq))
    with_bk = bool(np.any(bk))
    with_bv = bool(np.any(bv))
    with_bo = bool(np.any(bo))
    cfg = (with_mask, with_bq, with_bk, with_bv, with_bo)
    nc = _get_nc(cfg)

    scale = np.float32(1.0 / np.sqrt(D))
    WqT = np.ascontiguousarray((Wq.T * scale)).astype(BF16_NP)
    WkT = np.ascontiguousarray(Wk.T).astype(BF16_NP)
    WvT = np.ascontiguousarray(Wv.T).astype(BF16_NP)
    WoT = np.ascontiguousarray(Wo.T).astype(BF16_NP)
    ones_h = np.ones((P, H), BF16_NP)

    base = {"WqT": WqT, "WkT": WkT, "WvT": WvT, "WoT": WoT, "ones_h": ones_h}
    if with_bq:
        base["bq2"] = np.ascontiguousarray((bq * scale).reshape(HD, 1))
    if with_bk:
        base["bk2"] = np.ascontiguousarray(bk.reshape(HD, 1))
    if with_bv:
        bv_ext = np.zeros((1, H * 65), np.float32)
        bv_ext[0, :].reshape(H, 65)[:, 0:64] = bv.reshape(H, 64)
        base["bv_ext"] = bv_ext
    if with_bo:
        base["bo2"] = np.ascontiguousarray(bo.reshape(D, 1))
    if with_mask:
        base["maskT"] = np.ascontiguousarray(
            np.where(mask == 0, np.float32(NEG), np.float32(0.0)).astype(np.float32).T)

    in_maps = []
    for b in range(B):
        m = dict(base)
        m["qT"] = q[b].T.astype(BF16_NP)
        m["kT"] = k[b].T.astype(BF16_NP)
        m["vT"] = v[b].T.astype(BF16_NP)
        in_maps.append(m)

    res = run_bass_kernel_spmd(nc, in_maps, core_ids=list(range(B)))
    LAST_RESULTS = res

    out = np.stack([r["out_t"].T for r in res.results]).astype(np.float32)  # (B, T, D)
    attn = np.empty((B, H, T, T), np.float32)
    for b in range(B):
        # scratch rows: (h, c, p) x cols (kt, n); attn[h, tq=c*TQC+n, tk=kt*P+p]
        s = res.results[b]["attn_s"].reshape(H, NCH, P, NT, TQC)
        attn[b] = s.transpose(0, 1, 4, 3, 2).astype(np.float32).reshape(H, T, T)
    return out, attn


# revision 14
# speedup vs baseline: 1.1628x; 1.1628x over previous
"""Multi-head attention (B=8, T=1024, d_model=1024, H=16, d=64) on 8 trn2 cores.

Strategy: data-parallel over batch — one batch element per NeuronCore, no
collectives. Per core, everything is computed in "transposed" layouts so that
every matmul has its contraction on the partition dim and a 512-wide moving
operand:

  qhT/khT = (Wq.T @ q.T) etc.            [hd, t]   (PE, bf16 in / fp32 psum)
  vh      = v @ Wv.T (+ ones column)     [t, hd]   per-head [tk, 64+1]
  S.T     = khT_h.T @ qhT_h              [tk, tq]  (K=64)
  E       = exp(S.T)                     (ACT, no max-subtract: logits O(6))
  ctx_ext = [vh | 1].T @ E               [65, tq]  row 64 = softmax denominators
  attn.T  = E * (1/denom)                (DVE 4x bf16, partition-broadcast recip)
  out.T   = Wo @ (ctx_ext[0:64]/denom)   [64, t]   fp32 out

attn is written to HBM as one contiguous [128, 4096] block per (head, tq-chunk)
job; the host unscrambles to (B,H,Tq,Tk) fp32. The matmul datapath is bf16
(fp32 PSUM accumulation) which enables FWL fast weight loads and 4x DVE modes.
"""
import os
import sys
from contextlib import ExitStack

import numpy as np
import ml_dtypes

for _p in ("/opt/trn_rl_repo", os.path.expanduser("~/.axon_site/_ro/trn_rl_repo")):
    if os.path.isdir(_p) and _p not in sys.path:
        sys.path.append(_p)

import concourse.bass as bass  # noqa: E402
import concourse.tile as tile  # noqa: E402
from concourse import bacc, mybir  # noqa: E402
from concourse.bass_utils import run_bass_kernel_spmd  # noqa: E402

# Route every activation to the one table that holds Exp+Ln+Copy+Identity
# (all funcs this kernel uses). The default chooser takes the first table
# containing each func (Exp->exp_and_others, Ln->natural_log), which forces
# a ~1.3us ACT table reload per alternation. Emptying the other sets keeps
# list positions (= act_func_set_id) intact while making the choice unique.
_orig_gat = bacc.get_activation_tables


def _gat_single_table(arch):
    t = _orig_gat(arch)
    keep = "natural_log_exp_and_others"
    if keep in t:
        return {name: (s if name == keep else set()) for name, s in t.items()}
    return t


bacc.get_activation_tables = _gat_single_table

F32 = mybir.dt.float32
BF16 = mybir.dt.bfloat16
AF = mybir.ActivationFunctionType
BF16_NP = ml_dtypes.bfloat16

B, T, DM, H, D = 8, 1024, 1024, 16, 64
HD = H * D
P = 128
NT = T // P     # 8 partition tiles along t
NK = DM // P    # 8 contraction tiles along d_model / hd
TQC = 512       # tq chunk (one PSUM bank)
NCH = T // TQC  # 2
NEG = -1.0e9
E_BUFS = 4      # merged exp tiles [128, 2*NT*TQC]: jobs in flight

LAST_RESULTS = None


def _act_recip_lnexp(nc, out, tmp, in_):
    """1/x as exp(-ln(x)) on ACT — both funcs live in the
    natural_log_exp_and_others table, so no ACT table reloads (unlike
    Reciprocal, which forces a table swap against Exp every job)."""
    nc.scalar.activation(tmp, in_, AF.Ln)
    nc.scalar.activation(out, tmp, AF.Exp, scale=-1.0)


def _build(with_mask, with_bq, with_bk, with_bv, with_bo):
    nc = bacc.Bacc("TRN2", target_bir_lowering=False, debug=False, num_devices=1)

    qT_d = nc.dram_tensor("qT", [DM, T], BF16, kind="ExternalInput")
    kT_d = nc.dram_tensor("kT", [DM, T], BF16, kind="ExternalInput")
    vT_d = nc.dram_tensor("vT", [DM, T], BF16, kind="ExternalInput")
    wqT_d = nc.dram_tensor("WqT", [DM, HD], BF16, kind="ExternalInput")
    wkT_d = nc.dram_tensor("WkT", [DM, HD], BF16, kind="ExternalInput")
    wvT_d = nc.dram_tensor("WvT", [DM, HD], BF16, kind="ExternalInput")
    woT_d = nc.dram_tensor("WoT", [HD, D], BF16, kind="ExternalInput")
    ones_d = nc.dram_tensor("ones_h", [P, H], BF16, kind="ExternalInput")
    bq_d = nc.dram_tensor("bq2", [HD, 1], F32, kind="ExternalInput") if with_bq else None
    bk_d = nc.dram_tensor("bk2", [HD, 1], F32, kind="ExternalInput") if with_bk else None
    bv_d = nc.dram_tensor("bv_ext", [1, H * 65], F32, kind="ExternalInput") if with_bv else None
    bo_d = nc.dram_tensor("bo2", [D, 1], F32, kind="ExternalInput") if with_bo else None
    maskT_d = nc.dram_tensor("maskT", [T, T], F32, kind="ExternalInput") if with_mask else None

    # per job (h, c): rows (h*NCH+c)*128 .. +128, cols = (kt, n) flattened
    attn_d = nc.dram_tensor("attn_s", [H * NCH * P, NT * TQC], BF16, kind="ExternalOutput")
    out_d = nc.dram_tensor("out_t", [D, T], F32, kind="ExternalOutput")

    with tile.TileContext(nc) as tc, ExitStack() as ctx, \
         nc.allow_low_precision(reason="bf16 attention datapath by design"):
        # ---- persistent pools -------------------------------------------
        qhT_p = ctx.enter_context(tc.tile_pool(name="qhT", bufs=NK))
        khT_p = ctx.enter_context(tc.tile_pool(name="khT", bufs=NK))
        vh_p = ctx.enter_context(tc.tile_pool(name="vh", bufs=NT))
        ctx_p = ctx.enter_context(tc.tile_pool(name="ctxp", bufs=NK))
        wo_p = ctx.enter_context(tc.tile_pool(name="wo", bufs=NK))
        sm_p = ctx.enter_context(tc.tile_pool(name="smallp", bufs=4))

        qhT = [qhT_p.tile([P, T], BF16, tag="qhT", name=f"qhT{i}") for i in range(NK)]
        khT = [khT_p.tile([P, T], BF16, tag="khT", name=f"khT{i}") for i in range(NK)]
        vh = [vh_p.tile([P, H * 65], BF16, tag="vh", name=f"vh{i}") for i in range(NT)]
        ctxT = [ctx_p.tile([P, T], BF16, tag="ctx", name=f"ctxT{i}") for i in range(NK)]
        wo_sb = [wo_p.tile([P, D], BF16, tag="wo", name=f"wo{i}") for i in range(NK)]
        for k in range(NK):
            nc.sync.dma_start(wo_sb[k][:], woT_d.ap()[k * P:(k + 1) * P, :])

        # ones columns of vh (slot 64 of each per-head 65-block)
        for i in range(NT):
            dst = vh[i][:, :].rearrange("p (h x) -> p h x", x=65)[:, :, 64:65]
            src = ones_d.ap()[:, :].rearrange("p (h o) -> p h o", o=1)
            nc.sync.dma_start(dst, src)

        bo_sb = None
        if with_bo:
            bo_sb = sm_p.tile([D, 1], F32, tag="bo")
            nc.sync.dma_start(bo_sb[:], bo_d.ap())

        # ---- phases P (projections) + A (attention), interleaved --------
        with ExitStack() as actx:
            w_p = actx.enter_context(tc.tile_pool(name="wf", bufs=NK + 2))
            x_p = actx.enter_context(tc.tile_pool(name="xf", bufs=NK + 2))
            bias_p = actx.enter_context(tc.tile_pool(name="biasp", bufs=2 * NK + 2))
            e_p = actx.enter_context(tc.tile_pool(name="e", bufs=E_BUFS))
            b_p = actx.enter_context(tc.tile_pool(name="bb", bufs=3))
            r_p = actx.enter_context(tc.tile_pool(name="rc", bufs=4))
            # [128,1024] 2-bank tiles shared by projection groups and score
            # pairs (3x2 banks) + ctx accumulators (2 banks) = 8 banks.
            ps_s = actx.enter_context(tc.tile_pool(name="pss", bufs=3, space="PSUM"))
            ps_c = actx.enter_context(tc.tile_pool(name="psc", bufs=2, space="PSUM"))
            maskT_sb = None
            if with_mask:
                m_p = actx.enter_context(tc.tile_pool(name="maskp", bufs=NT))
                maskT_sb = []
                for kt in range(NT):
                    mt = m_p.tile([P, T], F32, tag="mask", name=f"mask_{kt}")
                    nc.sync.dma_start(mt[:], maskT_d.ap()[kt * P:(kt + 1) * P, :])
                    maskT_sb.append(mt)

            def load_full(pool, tag, d_tensor, pname):
                ts = []
                for k in range(NK):
                    t_ = pool.tile([P, T], BF16, tag=tag, name=f"{pname}_{k}")
                    nc.sync.dma_start(t_[:], d_tensor.ap()[k * P:(k + 1) * P, :])
                    ts.append(t_)
                return ts

            def load_bias(b_d, pname):
                b_sb = []
                for i in range(NK):
                    bt = bias_p.tile([P, 1], F32, tag="bias", name=f"bias{pname}_{i}")
                    nc.sync.dma_start(bt[:], b_d.ap()[i * P:(i + 1) * P, :])
                    b_sb.append(bt)
                return b_sb

            def proj_group_qk(w_sb, x_sb, dst_tiles, b_sb, i, pname):
                # both tq chunks of output tile i in one 2-bank psum tile,
                # then a single DVE copy to SBUF
                ps = ps_s.tile([P, 2 * TQC], F32, tag="sp", name=f"ps{pname}_{i}")
                for c in range(NCH):
                    for k in range(NK):
                        nc.tensor.matmul(
                            ps[:, c * TQC:(c + 1) * TQC],
                            w_sb[k][:, i * P:(i + 1) * P],
                            x_sb[k][:, c * TQC:(c + 1) * TQC],
                            start=(k == 0), stop=(k == NK - 1))
                if b_sb is not None:
                    nc.scalar.activation(dst_tiles[i][:], ps[:], AF.Identity, bias=b_sb[i][:])
                else:
                    nc.vector.tensor_copy(dst_tiles[i][:], ps[:])

            def proj_group_v(w_sb, x_sb, i):
                ps = ps_s.tile([P, 2 * TQC], F32, tag="sp", name=f"psv_{i}")
                for c in range(NCH):
                    for k in range(NK):
                        nc.tensor.matmul(
                            ps[:, c * TQC:(c + 1) * TQC],
                            x_sb[k][:, i * P:(i + 1) * P],
                            w_sb[k][:, c * TQC:(c + 1) * TQC],
                            start=(k == 0), stop=(k == NK - 1))
                dst = vh[i][:, :].rearrange("p (h x) -> p h x", x=65)[:, :, 0:64]
                src = ps[:, :].rearrange("p (h x) -> p h x", x=64)
                nc.vector.tensor_copy(dst, src)

            NTQ = NT * TQC

            def scores(hp, c):
                # heads (2hp, 2hp+1) share khT/qhT tile hp; the two K=64
                # matmuls are row-packed (tile_position) and run concurrently,
                # writing the two halves of one 2-bank psum tile so a single
                # [128,1024] exp serves both heads.
                eb = e_p.tile([P, 2 * NTQ], BF16, tag="e", name=f"e_{hp}_{c}")
                for kt in range(NT):
                    sp = ps_s.tile([P, 2 * TQC], F32, tag="sp", name=f"sp_{hp}_{c}_{kt}")
                    for s_i in range(2):
                        nc.tensor.matmul(
                            sp[:, s_i * TQC:(s_i + 1) * TQC],
                            khT[hp][s_i * 64:(s_i + 1) * 64, kt * P:(kt + 1) * P],
                            qhT[hp][s_i * 64:(s_i + 1) * 64, c * TQC:(c + 1) * TQC],
                            start=True, stop=True, tile_position=(s_i * 64, 0))
                    if with_mask:
                        mrep = maskT_sb[kt][:, c * TQC:(c + 1) * TQC] \
                            .rearrange("p (o q) -> p o q", o=1).to_broadcast([P, 2, TQC])
                        nc.vector.tensor_add(
                            sp[:, :].rearrange("p (s q) -> p s q", s=2),
                            sp[:, :].rearrange("p (s q) -> p s q", s=2), mrep)
                    dst = eb[:, :].rearrange("p (s q) -> p s q", q=NTQ)[:, :, kt * TQC:(kt + 1) * TQC]
                    nc.scalar.activation(dst, sp[:, :].rearrange("p (s q) -> p s q", s=2), AF.Exp)
                return eb

            def tail(hp, c, eb):
                for s_i in range(2):
                    h = 2 * hp + s_i
                    off = s_i * NTQ
                    cp = ps_c.tile([D + 1, TQC], F32, tag="cp", name=f"cp_{h}_{c}")
                    for kt in range(NT):
                        nc.tensor.matmul(
                            cp[:], vh[kt][:, h * 65:(h + 1) * 65],
                            eb[:, off + kt * TQC:off + (kt + 1) * TQC],
                            start=(kt == 0), stop=(kt == NT - 1))
                    # 1/denominators straight off the psum row (ACT, bf16 out)
                    lt = r_p.tile([1, TQC], F32, tag="lt", name=f"lt_{h}_{c}")
                    rc = r_p.tile([1, TQC], BF16, tag="rc", name=f"rc_{h}_{c}")
                    _act_recip_lnexp(nc, rc[:], lt[:], cp[D:D + 1, :])
                    bb = b_p.tile([P, TQC], BF16, tag="bb", name=f"bb_{h}_{c}")
                    nc.gpsimd.partition_broadcast(bb[:], rc[:])
                    # ctx normalize (psum fp32 x bf16 -> bf16)
                    nc.vector.tensor_mul(
                        ctxT[hp][s_i * 64:(s_i + 1) * 64, c * TQC:(c + 1) * TQC],
                        cp[0:D, :], bb[0:D, :])
                    # attn normalize: one mul over the head's half of eb
                    bb_rep = bb[:, :].rearrange("p (k n) -> p k n", k=1).to_broadcast([P, NT, TQC])
                    nc.vector.tensor_mul(
                        eb[:, off:off + NTQ].rearrange("p (k n) -> p k n", n=TQC),
                        eb[:, off:off + NTQ].rearrange("p (k n) -> p k n", n=TQC), bb_rep)
                    row = (h * NCH + c) * P
                    nc.sync.dma_start(attn_d.ap()[row:row + P, :], eb[:, off:off + NTQ])

            jobs = [(hp, c) for hp in range(H // 2) for c in range(NCH)]
            pend = []

            # q projection
            wq_sb = load_full(w_p, "wf", wqT_d, "qw")
            xq_sb = load_full(x_p, "xf", qT_d, "qx")
            bq_sb = load_bias(bq_d, "q") if bq_d is not None else None
            for i in range(NK):
                proj_group_qk(wq_sb, xq_sb, qhT, bq_sb, i, "q")

            # k projection with the first scores jobs interleaved (khT[hp]
            # is ready after group hp, so exps start ~40us earlier)
            wk_sb = load_full(w_p, "wf", wkT_d, "kw")
            xk_sb = load_full(x_p, "xf", kT_d, "kx")
            bk_sb = load_bias(bk_d, "k") if bk_d is not None else None
            for i in range(NK):
                proj_group_qk(wk_sb, xk_sb, khT, bk_sb, i, "k")
                if i in (1, 4, 7):
                    j = jobs[len(pend)]
                    pend.append((j, scores(*j)))

            # v projection
            wv_sb = load_full(w_p, "wf", wvT_d, "vw")
            xv_sb = load_full(x_p, "xf", vT_d, "vx")
            bvb = None
            if with_bv:
                bv_row = bias_p.tile([1, H * 65], F32, tag="bvrow")
                nc.sync.dma_start(bv_row[:], bv_d.ap())
                bvb = bias_p.tile([P, H * 65], F32, tag="bvb")
                nc.gpsimd.partition_broadcast(bvb[:], bv_row[:])
            for i in range(NT):
                proj_group_v(wv_sb, xv_sb, i)
                if with_bv:
                    nc.vector.tensor_add(vh[i][:], vh[i][:], bvb[:])

            # steady-state software pipeline over the remaining jobs
            for j in jobs[len(pend):]:
                pj, peb = pend.pop(0)
                tail(pj[0], pj[1], peb)
                pend.append((j, scores(*j)))
            for pj, peb in pend:
                tail(pj[0], pj[1], peb)

        # ---- phase O: output projection ---------------------------------
        with tc.tile_pool(name="pso", bufs=2, space="PSUM") as ps_o, \
             tc.tile_pool(name="ot", bufs=1) as ot_p:
            outT = ot_p.tile([D, T], F32)
            for c in range(NCH):
                op = ps_o.tile([D, TQC], F32, tag="op", name=f"op_{c}")
                for k in range(NK):
                    nc.tensor.matmul(
                        op[:], wo_sb[k][:], ctxT[k][:, c * TQC:(c + 1) * TQC],
                        start=(k == 0), stop=(k == NK - 1))
                if with_bo:
                    nc.scalar.activation(outT[:, c * TQC:(c + 1) * TQC], op[:], AF.Identity, bias=bo_sb[:])
                else:
                    nc.scalar.activation(outT[:, c * TQC:(c + 1) * TQC], op[:], AF.Copy)
            nc.sync.dma_start(out_d.ap(), outT[:])

    nc.compile()
    return nc


_NC_CACHE = {}


def _get_nc(cfg):
    if cfg not in _NC_CACHE:
        _NC_CACHE[cfg] = _build(*cfg)
    return _NC_CACHE[cfg]


def kernel(q, k, v, Wq, bq, Wk, bk, Wv, bv, Wo, bo, mask):
    global LAST_RESULTS
    q = np.asarray(q, np.float32)
    k = np.asarray(k, np.float32)
    v = np.asarray(v, np.float32)
    Wq = np.asarray(Wq, np.float32)
    Wk = np.asarray(Wk, np.float32)
    Wv = np.asarray(Wv, np.float32)
    Wo = np.asarray(Wo, np.float32)
    bq = np.asarray(bq, np.float32)
    bk = np.asarray(bk, np.float32)
    bv = np.asarray(bv, np.float32)
    bo = np.asarray(bo, np.float32)
    mask = np.asarray(mask)
    assert q.shape == (B, T, DM) and k.shape == (B, T, DM) and v.shape == (B, T, DM)

    with_mask = bool((mask == 0).any())
    with_bq = bool(np.any(bq))
    with_bk = bool(np.any(bk))
    with_bv = bool(np.any(bv))
    with_bo = bool(np.any(bo))
    cfg = (with_mask, with_bq, with_bk, with_bv, with_bo)
    nc = _get_nc(cfg)

    scale = np.float32(1.0 / np.sqrt(D))
    WqT = np.ascontiguousarray((Wq.T * scale)).astype(BF16_NP)
    WkT = np.ascontiguousarray(Wk.T).astype(BF16_NP)
    WvT = np.ascontiguousarray(Wv.T).astype(BF16_NP)
    WoT = np.ascontiguousarray(Wo.T).astype(BF16_NP)
    ones_h = np.ones((P, H), BF16_NP)

    base = {"WqT": WqT, "WkT": WkT, "WvT": WvT, "WoT": WoT, "ones_h": ones_h}
    if with_bq:
        base["bq2"] = np.ascontiguousarray((bq * scale).reshape(HD, 1))
    if with_bk:
        base["bk2"] = np.ascontiguousarray(bk.reshape(HD, 1))
    if with_bv:
        bv_ext = np.zeros((1, H * 65), np.float32)
        bv_ext[0, :].reshape(H, 65)[:, 0:64] = bv.reshape(H, 64)
        base["bv_ext"] = bv_ext
    if with_bo:
        base["bo2"] = np.ascontiguousarray(bo.reshape(D, 1))
    if with_mask:
        base["maskT"] = np.ascontiguousarray(
            np.where(mask == 0, np.float32(NEG), np.float32(0.0)).astype(np.float32).T)

    in_maps = []
    for b in range(B):
        m = dict(base)
        m["qT"] = q[b].T.astype(BF16_NP)
        m["kT"] = k[b].T.astype(BF16_NP)
        m["vT"] = v[b].T.astype(BF16_NP)
        in_maps.append(m)

    res = run_bass_kernel_spmd(nc, in_maps, core_ids=list(range(B)))
    LAST_RESULTS = res

    out = np.stack([r["out_t"].T for r in res.results]).astype(np.float32)  # (B, T, D)
    attn = np.empty((B, H, T, T), np.float32)
    for b in range(B):
        # scratch rows: (h, c, p) x cols (kt, n); attn[h, tq=c*TQC+n, tk=kt*P+p]
        s = res.results[b]["attn_s"].reshape(H, NCH, P, NT, TQC)
        attn[b] = s.transpose(0, 1, 4, 3, 2).astype(np.float32).reshape(H, T, T)
    return out, attn
